# revision 22
# baseline (speedup 1.0000x reference)
"""TRN2 Bass kernel for nn_Attention_5720896438407 (8-core data-parallel), v10.

Math (see v2 docstring): attention collapses to
    Y = s * (x @ Wf.T @ M.T) + bias-terms,   s = rsqrt(mean(T^2) + eps),
    T = x @ kv_a_w.T   (needed only for stats -> fp8 DoubleRow at 2x rate)
with Wf = (kv_b_w[128:]*(1+kv_norm_w)) @ kv_a_w and M = head-sum of o_w,
both folded on the host.

v11 structure (from v8/v9/v10 traces: input wire is ~0.35 MiB/us per core
and the kernel is PE-work bound once fed; the head needs real work to
chew while the value-path tensors stream in; Pool casts cost 7us/slab so
all fp8 stats data comes from the host):
- x16 is shipped in token quarters (256 tok x all 16 chunks, 1 MiB each);
  vraw runs per quarter (16 MMs at N=256, ~100ns each) so the tail only
  waits on the last 1 MiB piece.
- stats slabs 0/1 start on the first two pieces (w8 quarter + x8s01) with
  chunk-split emission; stats 2/3 fill the PE window while wf/xq0 stream.
- 12 junk matmuls bridge the preamble to first data and hold the HAM
  clock gate at 8/8.
- Outputs: slabs 0-4 ride the scalar HWDGE ring (so they do not queue
  behind the input FIFO on sync); tail slabs 5-7 ride sync in column
  halves once the input stream has drained.
"""
import sys

sys.path.insert(0, "/opt/trn_rl_repo")

import numpy as np
import ml_dtypes
import concourse.bass as bass
import concourse.tile as tile
from concourse import bacc, mybir
from concourse.bass_utils import run_bass_kernel_spmd

F32 = mybir.dt.float32
F16 = mybir.dt.float16
F8 = mybir.dt.float8e4
DR = mybir.MatmulPerfMode.DoubleRow
AF = mybir.ActivationFunctionType

HID = 2048
KV = 512
D = 128
OUT = 2048
EPS = 1e-6
NCK = HID // 128         # 16 hid chunks
SLAB = 128               # tokens per stats slab
QGRP = 256               # tokens per value-path quarter group
N_CORES = 8
WSCALE = 64.0            # host pre-scale on kv_a_w so fp8 stays normal
NJUNK = 13               # warmup matmuls (N=512): HAM flip + bridge to data
NHOST8 = 8               # all-host fp8: on-chip casts poison PE via SBUF contention

_NC_CACHE = {}


def _build_nc(tok, with_ba, with_bv):
    nslab = tok // SLAB        # 8
    nq = tok // QGRP           # 4
    assert tok % QGRP == 0 and QGRP % SLAB == 0

    nc = bacc.Bacc("TRN2", target_bir_lowering=False, debug=False,
                   num_devices=1)

    x8_d = nc.dram_tensor("x8", (NHOST8 // 2, 128, 2, NCK, SLAB), F8,
                          kind="ExternalInput").ap()
    x16_d = nc.dram_tensor("x16", (nq, 128, NCK, QGRP), F16,
                           kind="ExternalInput").ap()
    w8_d = nc.dram_tensor("w8", (128, NCK, KV), F8, kind="ExternalInput").ap()
    wf_d = nc.dram_tensor("wf", (128, NCK, D), F16, kind="ExternalInput").ap()
    mt_d = nc.dram_tensor("mt", (D, OUT), F16, kind="ExternalInput").ap()
    if with_ba:
        ba_d = nc.dram_tensor("bar", (1, KV), F8, kind="ExternalInput").ap()
        ones8_d = nc.dram_tensor("ones8", (1, 128), F8,
                                 kind="ExternalInput").ap()
        vb_d = nc.dram_tensor("vb", (128, 1), F32, kind="ExternalInput").ap()
    if with_bv:
        crb_d = nc.dram_tensor("crb", (128, OUT), F16,
                               kind="ExternalInput").ap()
    y_d = nc.dram_tensor("y", (tok, OUT), F16, kind="ExternalOutput").ap()

    with tile.TileContext(nc) as tc:
        with tc.tile_pool(name="consts", bufs=1) as consts, \
             tc.tile_pool(name="xs8", bufs=1) as xs8, \
             tc.tile_pool(name="xs16", bufs=1) as xs16, \
             tc.tile_pool(name="work", bufs=2) as work, \
             tc.tile_pool(name="ps_t", bufs=2, space="PSUM") as ps_t, \
             tc.tile_pool(name="ps_v", bufs=2, space="PSUM") as ps_v, \
             tc.tile_pool(name="ps_y", bufs=4, space="PSUM") as ps_y:
            w8_s = consts.tile([128, NCK, KV], F8, tag="w8")
            x8_s = xs8.tile([128, nslab, NCK, SLAB], F8, tag="x8")
            x16_s = [xs16.tile([128, NCK, QGRP], F16, tag=f"x16_{q}",
                               name=f"x16_{q}") for q in range(nq)]
            wf_s = consts.tile([128, NCK, D], F16, tag="wf")
            mt_s = consts.tile([128, OUT], F16, tag="mt")
            # sync ring: strict FIFO; pieces ordered by PE need-by time at
            # the measured ~0.35 MiB/us wire rate (first piece ~11us).
            # x8s01 rides the scalar HWDGE ring so it lands concurrently
            # with w8a instead of serializing behind it on sync.
            nc.scalar.dma_start(x8_s[:, 0:2, :, :], x8_d[0])
            nc.sync.dma_start(w8_s[:, 0:4, :], w8_d[:, 0:4, :])
            nc.sync.dma_start(w8_s[:, 4:10, :], w8_d[:, 4:10, :])
            nc.sync.dma_start(w8_s[:, 10:16, :], w8_d[:, 10:16, :])
            nc.sync.dma_start(x8_s[:, 2:4, :, :], x8_d[1])
            nc.sync.dma_start(wf_s[:], wf_d)
            nc.sync.dma_start(x16_s[0][:], x16_d[0])
            nc.sync.dma_start(mt_s[:], mt_d)
            nc.sync.dma_start(x16_s[1][:], x16_d[1])
            nc.sync.dma_start(x8_s[:, 4:6, :, :], x8_d[2])
            nc.sync.dma_start(x16_s[2][:], x16_d[2])
            nc.sync.dma_start(x8_s[:, 6:8, :, :], x8_d[3])
            nc.sync.dma_start(x16_s[3][:], x16_d[3])
            if with_ba:
                ba_s = consts.tile([1, KV], F8, tag="ba")
                nc.scalar.dma_start(ba_s[:], ba_d)
                ones8_s = consts.tile([1, 128], F8, tag="ones8")
                nc.scalar.dma_start(ones8_s[:], ones8_d)
                vb_s = consts.tile([128, 1], F32, tag="vb")
                nc.scalar.dma_start(vb_s[:], vb_d)
            if with_bv:
                crb_s = consts.tile([128, OUT], F16, tag="crb")
                nc.scalar.dma_start(crb_s[:], crb_d)

            # ---------------- local consts + PE warm-up ----------------
            eps_s = consts.tile([128, 1], F32, tag="eps")
            nc.vector.memset(eps_s[:], EPS)
            jsa = consts.tile([128, 128], F16, tag="jsa")
            nc.vector.memset(jsa[:], 0.0)
            jsb = consts.tile([128, 512], F16, tag="jsb")
            nc.vector.memset(jsb[:], 0.0)
            # preload Square/Sqrt activation tables off the critical path
            tl = consts.tile([128, 1], F32, tag="tl")
            nc.scalar.activation(tl[:], eps_s[:], AF.Square)
            nc.scalar.activation(tl[:], eps_s[:], AF.Sqrt)
            for i in range(NJUNK):
                junk = ps_y.tile([128, 512], F32, tag="py", name=f"junk{i}")
                nc.tensor.matmul(junk[:], jsa[:], jsb[:],
                                 start=True, stop=True)

            # ---------------- per-stage bodies ----------------
            sq_sc = work.tile([128, KV], F16, tag="sq", bufs=2)
            vts = consts.tile([128, tok], F16, tag="vts")
            s_t = [None] * nslab
            pt_t = [None] * nslab

            def stats_mm(g, ka, kb):
                if pt_t[g] is None:
                    pt_t[g] = ps_t.tile([128, KV], F32, tag="pt",
                                        name=f"pt{g}")
                pt = pt_t[g]
                for k in range(ka, kb):
                    nc.tensor.matmul(pt[:], x8_s[:, g, 2 * k:2 * k + 2, :],
                                     w8_s[:, 2 * k:2 * k + 2, :],
                                     start=(k == 0),
                                     stop=(k == 7 and not with_ba),
                                     perf_mode=DR)

            def stats_fin(g):
                pt = pt_t[g]
                if with_ba:
                    nc.tensor.matmul(pt[:], ones8_s[:], ba_s[:],
                                     start=False, stop=True)
                ssq = work.tile([128, 1], F32, tag="ssq")
                nc.scalar.activation(sq_sc[:], pt[:], AF.Square,
                                     accum_out=ssq[:])
                rt = work.tile([128, 1], F32, tag="rt")
                nc.scalar.activation(rt[:], ssq[:], AF.Sqrt, bias=eps_s[:],
                                     scale=1.0 / (KV * WSCALE * WSCALE))
                sg = work.tile([128, 1], F32, tag="sg", name=f"sg{g}", bufs=4)
                nc.vector.reciprocal(sg[:], rt[:])
                s_t[g] = sg

            def stats(g):
                stats_mm(g, 0, 8)
                stats_fin(g)

            def cast8(g):
                # fp16 -> fp8 stats copy for slab g from its x16 quarter.
                # Emission position matters: DVE is strict FIFO, so this
                # must be emitted only where its x16 piece is already due.
                q = g // 2
                t0 = (g % 2) * SLAB
                nc.vector.tensor_copy(x8_s[:, g, :, :],
                                      x16_s[q][:, :, t0:t0 + SLAB])

            def vraw(q):
                pv = ps_v.tile([128, 512], F32, tag="pv", name=f"pv{q}")
                for ck in range(NCK):
                    nc.tensor.matmul(pv[:, 0:QGRP], wf_s[:, ck, :],
                                     x16_s[q][:, ck, :],
                                     start=(ck == 0), stop=(ck == NCK - 1))
                dst = vts[:, q * QGRP:(q + 1) * QGRP]
                if with_ba:
                    nc.scalar.activation(dst, pv[:, 0:QGRP], AF.Identity,
                                         bias=vb_s[:], scale=1.0)
                else:
                    nc.vector.tensor_copy(dst, pv[:, 0:QGRP])

            def step4(g):
                t0 = g * SLAB
                ysb = work.tile([128, OUT], F16, tag="ysb", bufs=6)
                for n in range(4):
                    py = ps_y.tile([128, 512], F32, tag="py",
                                   name=f"py{g}_{n}")
                    nc.tensor.matmul(py[:], vts[:, t0:t0 + SLAB],
                                     mt_s[:, n * 512:(n + 1) * 512],
                                     start=True, stop=True)
                    ysl = ysb[:, n * 512:(n + 1) * 512]
                    if n % 2 == 0:
                        nc.vector.tensor_scalar_mul(ysl, py[:], s_t[g][:])
                    else:
                        nc.scalar.activation(ysl, py[:], AF.Identity,
                                             bias=0.0, scale=s_t[g][:])
                    if with_bv:
                        nc.vector.tensor_add(
                            ysl, ysl, crb_s[:, n * 512:(n + 1) * 512])
                    if g >= nslab - 3 and n == 1:
                        # tail slabs: first half fired as soon as it is
                        # scaled; earlier slabs ride scalar so they do
                        # not queue behind the input FIFO on sync
                        nc.sync.dma_start(y_d[t0:t0 + SLAB, 0:1024],
                                          ysb[:, 0:1024])
                if g >= nslab - 3:
                    nc.sync.dma_start(y_d[t0:t0 + SLAB, 1024:2048],
                                      ysb[:, 1024:2048])
                else:
                    nc.scalar.dma_start(y_d[t0:t0 + SLAB, :], ysb[:])

            # ---------------- PE emission order ----------------
            # slabs 0/1 staggered by w8 piece so PE starts on the first
            # 0.75 MiB; stats 2/3 fill the window while wf/xq0 stream in.
            stats_mm(0, 0, 2)
            stats_mm(1, 0, 2)
            stats_mm(0, 2, 5)
            stats_mm(1, 2, 5)
            stats_mm(0, 5, 8)
            stats_mm(1, 5, 8)
            stats_fin(0)
            stats_fin(1)
            stats(2)
            stats(3)
            vraw(0)
            step4(0)
            step4(1)
            vraw(1)
            step4(2)
            step4(3)
            stats(4)
            stats(5)
            vraw(2)
            step4(4)
            step4(5)
            stats(6)
            stats(7)
            vraw(3)
            step4(6)
            step4(7)

    nc.compile()
    return nc


def _host_prep(inputs):
    h = np.asarray(inputs["hidden_states"], dtype=np.float32)
    b, s, hid = h.shape
    assert hid == HID
    x = np.ascontiguousarray(h.reshape(b * s, hid))
    ntok = b * s
    tok = ntok // N_CORES
    nq = tok // QGRP

    kv_a_w = np.asarray(inputs["kv_a_w"], np.float64)
    kv_a_b = np.asarray(inputs["kv_a_b"], np.float64)
    kv_norm_w = np.asarray(inputs["kv_norm_w"], np.float64)
    kv_b_w = np.asarray(inputs["kv_b_w"], np.float64)
    kv_b_b = np.asarray(inputs["kv_b_b"], np.float64)
    o_w = np.asarray(inputs["o_w"], np.float64)

    wv = kv_b_w[D:2 * D] * (1.0 + kv_norm_w)[None, :]          # (128, 512)
    wf = wv @ kv_a_w                                           # (128, 2048)
    M = o_w.reshape(HID, 16, D).sum(axis=1)                    # (2048, 128)

    w8 = np.ascontiguousarray(
        (kv_a_w.T * WSCALE).reshape(NCK, 128, KV).transpose(1, 0, 2)
    ).astype(np.float32).astype(ml_dtypes.float8_e4m3)
    wf_sw = np.ascontiguousarray(
        wf.T.reshape(NCK, 128, D).transpose(1, 0, 2)).astype(np.float16)
    mt = np.ascontiguousarray(M.T).astype(np.float16)

    with_ba = bool(np.any(kv_a_b != 0.0))
    with_bv = bool(np.any(kv_b_b[D:2 * D] != 0.0))

    in_maps = []
    for i in range(N_CORES):
        shard = x[i * tok:(i + 1) * tok]                       # (tok, 2048)
        xt = shard.T.reshape(NCK, 128, tok)                    # hid-major
        ht = NHOST8 * SLAB
        x8 = np.ascontiguousarray(
            xt[:, :, :ht].reshape(NCK, 128, NHOST8 // 2, 2, SLAB)
            .transpose(2, 1, 3, 0, 4)
        ).astype(ml_dtypes.float8_e4m3)
        x16 = np.ascontiguousarray(
            xt.reshape(NCK, 128, nq, QGRP).transpose(2, 1, 0, 3)
        ).astype(np.float16)
        m = {"x8": x8, "x16": x16, "w8": w8, "wf": wf_sw, "mt": mt}
        if with_ba:
            m["bar"] = (kv_a_b.reshape(1, KV) * WSCALE).astype(
                np.float32).astype(ml_dtypes.float8_e4m3)
            m["ones8"] = np.ones((1, 128), np.float32).astype(
                ml_dtypes.float8_e4m3)
            m["vb"] = np.ascontiguousarray(
                (wv @ kv_a_b).reshape(D, 1)).astype(np.float32)
        if with_bv:
            cr = (M @ kv_b_b[D:2 * D]).reshape(1, OUT)
            m["crb"] = np.ascontiguousarray(
                np.broadcast_to(cr, (128, OUT))).astype(np.float16)
        in_maps.append(m)

    def gather(results):
        y = np.concatenate([r["y"] for r in results], axis=0)
        return np.ascontiguousarray(y.reshape(b, s, HID).astype(np.float32))

    return in_maps, gather, with_ba, with_bv, tok


def _run(inputs, trace=False, **spmd_kwargs):
    in_maps, gather, with_ba, with_bv, tok = _host_prep(inputs)
    key = (tok, with_ba, with_bv)
    if key not in _NC_CACHE:
        _NC_CACHE[key] = _build_nc(tok, with_ba, with_bv)
    nc = _NC_CACHE[key]
    res = run_bass_kernel_spmd(nc, in_maps, core_ids=list(range(N_CORES)),
                               trace=trace, **spmd_kwargs)
    return gather(res.results), res


def kernel(**inputs) -> np.ndarray:
    y, _ = _run(inputs, trace=False)
    return y


# revision 23
# speedup vs baseline: 1.0283x; 1.0283x over previous
"""TRN2 Bass kernel for nn_Attention_5720896438407 (8-core data-parallel), v18.

Math (see v2 docstring): attention collapses to
    Y = s * (x @ Wf.T @ M.T) + bias-terms,   s = rsqrt(mean(T^2) + eps),
    T = x @ kv_a_w.T   (needed only for stats -> fp8 DoubleRow at 2x rate)
with Wf = (kv_b_w[128:]*(1+kv_norm_w)) @ kv_a_w and M = head-sum of o_w,
both folded on the host.

v18 structure (from v8-v17 traces: input wire is ~0.35-0.4 MiB/us per
core, first piece lands ~11-13us, and the kernel is PE-work bound once
fed ~28us of matmul at 216ns/N=512; on-chip fp16->fp8 casts were tried
on Pool (7us/slab) and DVE (SBUF-port contention stretches PE matmuls
20%) -- all fp8 stats data therefore comes from the host):
- x16 is shipped in token quarters (256 tok x all 16 chunks, 1 MiB each);
  vraw runs per quarter (16 MMs at N=256, ~100ns each) so the tail only
  waits on the last 1 MiB piece.
- stats slabs 0/1 start on the first two pieces (w8 quarter + x8s01) with
  chunk-split emission; stats 2/3 fill the PE window while wf/xq0 stream.
- 13 junk matmuls bridge the preamble to first data and hold the HAM
  clock gate at 8/8 (any PE gap >3.4us re-throttles to 1.2 GHz).
- stats(4..7) are emitted BEFORE their stage's vraw so the rsqrt chain
  (ACT square/sqrt + DVE recip) clears the ACT queue while the PE chews
  vraw; otherwise s_g lands late and the tail py matmuls serialize
  behind their consumers.
- Outputs: slabs 0-4 ride the scalar HWDGE ring (so they do not queue
  behind the input FIFO on sync); tail slabs 5-7 ride sync in column
  halves once the input stream has drained.
"""
import sys

sys.path.insert(0, "/opt/trn_rl_repo")

import numpy as np
import ml_dtypes
import concourse.bass as bass
import concourse.tile as tile
from concourse import bacc, mybir
from concourse.bass_utils import run_bass_kernel_spmd

F32 = mybir.dt.float32
F16 = mybir.dt.float16
F8 = mybir.dt.float8e4
DR = mybir.MatmulPerfMode.DoubleRow
AF = mybir.ActivationFunctionType

HID = 2048
KV = 512
D = 128
OUT = 2048
EPS = 1e-6
NCK = HID // 128         # 16 hid chunks
SLAB = 128               # tokens per stats slab
QGRP = 256               # tokens per value-path quarter group
N_CORES = 8
WSCALE = 64.0            # host pre-scale on kv_a_w so fp8 stays normal
NJUNK = 13               # warmup matmuls (N=512): HAM flip + bridge to data
NHOST8 = 8               # all-host fp8: on-chip casts poison PE via SBUF contention

_NC_CACHE = {}


def _build_nc(tok, with_ba, with_bv):
    nslab = tok // SLAB        # 8
    nq = tok // QGRP           # 4
    assert tok % QGRP == 0 and QGRP % SLAB == 0

    nc = bacc.Bacc("TRN2", target_bir_lowering=False, debug=False,
                   num_devices=1)

    x8_d = nc.dram_tensor("x8", (NHOST8 // 2, 128, 2, NCK, SLAB), F8,
                          kind="ExternalInput").ap()
    x16_d = nc.dram_tensor("x16", (nq, 128, NCK, QGRP), F16,
                           kind="ExternalInput").ap()
    w8_d = nc.dram_tensor("w8", (128, NCK, KV), F8, kind="ExternalInput").ap()
    wf_d = nc.dram_tensor("wf", (128, NCK, D), F16, kind="ExternalInput").ap()
    mt_d = nc.dram_tensor("mt", (D, OUT), F16, kind="ExternalInput").ap()
    if with_ba:
        ba_d = nc.dram_tensor("bar", (1, KV), F8, kind="ExternalInput").ap()
        ones8_d = nc.dram_tensor("ones8", (1, 128), F8,
                                 kind="ExternalInput").ap()
        vb_d = nc.dram_tensor("vb", (128, 1), F32, kind="ExternalInput").ap()
    if with_bv:
        crb_d = nc.dram_tensor("crb", (128, OUT), F16,
                               kind="ExternalInput").ap()
    y_d = nc.dram_tensor("y", (tok, OUT), F16, kind="ExternalOutput").ap()

    with tile.TileContext(nc) as tc:
        with tc.tile_pool(name="consts", bufs=1) as consts, \
             tc.tile_pool(name="xs8", bufs=1) as xs8, \
             tc.tile_pool(name="xs16", bufs=1) as xs16, \
             tc.tile_pool(name="work", bufs=2) as work, \
             tc.tile_pool(name="ps_t", bufs=2, space="PSUM") as ps_t, \
             tc.tile_pool(name="ps_v", bufs=2, space="PSUM") as ps_v, \
             tc.tile_pool(name="ps_y", bufs=4, space="PSUM") as ps_y:
            w8_s = consts.tile([128, NCK, KV], F8, tag="w8")
            x8_s = xs8.tile([128, nslab, NCK, SLAB], F8, tag="x8")
            x16_s = [xs16.tile([128, NCK, QGRP], F16, tag=f"x16_{q}",
                               name=f"x16_{q}") for q in range(nq)]
            wf_s = consts.tile([128, NCK, D], F16, tag="wf")
            mt_s = consts.tile([128, OUT], F16, tag="mt")
            # sync ring: strict FIFO; pieces ordered by PE need-by time at
            # the measured ~0.35 MiB/us wire rate (first piece ~11us).
            # x8s01 rides the scalar HWDGE ring so it lands concurrently
            # with w8a instead of serializing behind it on sync.
            nc.scalar.dma_start(x8_s[:, 0:2, :, :], x8_d[0])
            nc.sync.dma_start(w8_s[:, 0:4, :], w8_d[:, 0:4, :])
            nc.sync.dma_start(w8_s[:, 4:10, :], w8_d[:, 4:10, :])
            nc.sync.dma_start(w8_s[:, 10:16, :], w8_d[:, 10:16, :])
            nc.sync.dma_start(x8_s[:, 2:4, :, :], x8_d[1])
            nc.sync.dma_start(wf_s[:], wf_d)
            nc.sync.dma_start(x16_s[0][:], x16_d[0])
            nc.sync.dma_start(mt_s[:], mt_d)
            nc.sync.dma_start(x16_s[1][:], x16_d[1])
            nc.sync.dma_start(x8_s[:, 4:6, :, :], x8_d[2])
            nc.sync.dma_start(x16_s[2][:], x16_d[2])
            nc.sync.dma_start(x8_s[:, 6:8, :, :], x8_d[3])
            nc.sync.dma_start(x16_s[3][:], x16_d[3])
            if with_ba:
                ba_s = consts.tile([1, KV], F8, tag="ba")
                nc.scalar.dma_start(ba_s[:], ba_d)
                ones8_s = consts.tile([1, 128], F8, tag="ones8")
                nc.scalar.dma_start(ones8_s[:], ones8_d)
                vb_s = consts.tile([128, 1], F32, tag="vb")
                nc.scalar.dma_start(vb_s[:], vb_d)
            if with_bv:
                crb_s = consts.tile([128, OUT], F16, tag="crb")
                nc.scalar.dma_start(crb_s[:], crb_d)

            # ---------------- local consts + PE warm-up ----------------
            eps_s = consts.tile([128, 1], F32, tag="eps")
            nc.vector.memset(eps_s[:], EPS)
            jsa = consts.tile([128, 128], F16, tag="jsa")
            nc.vector.memset(jsa[:], 0.0)
            jsb = consts.tile([128, 512], F16, tag="jsb")
            nc.vector.memset(jsb[:], 0.0)
            # preload Square/Sqrt activation tables off the critical path
            tl = consts.tile([128, 1], F32, tag="tl")
            nc.scalar.activation(tl[:], eps_s[:], AF.Square)
            nc.scalar.activation(tl[:], eps_s[:], AF.Sqrt)
            for i in range(NJUNK):
                junk = ps_y.tile([128, 512], F32, tag="py", name=f"junk{i}")
                nc.tensor.matmul(junk[:], jsa[:], jsb[:],
                                 start=True, stop=True)

            # ---------------- per-stage bodies ----------------
            sq_sc = work.tile([128, KV], F16, tag="sq", bufs=2)
            vts = consts.tile([128, tok], F16, tag="vts")
            s_t = [None] * nslab
            pt_t = [None] * nslab

            def stats_mm(g, ka, kb):
                if pt_t[g] is None:
                    pt_t[g] = ps_t.tile([128, KV], F32, tag="pt",
                                        name=f"pt{g}")
                pt = pt_t[g]
                for k in range(ka, kb):
                    nc.tensor.matmul(pt[:], x8_s[:, g, 2 * k:2 * k + 2, :],
                                     w8_s[:, 2 * k:2 * k + 2, :],
                                     start=(k == 0),
                                     stop=(k == 7 and not with_ba),
                                     perf_mode=DR)

            def stats_fin(g):
                pt = pt_t[g]
                if with_ba:
                    nc.tensor.matmul(pt[:], ones8_s[:], ba_s[:],
                                     start=False, stop=True)
                ssq = work.tile([128, 1], F32, tag="ssq")
                nc.scalar.activation(sq_sc[:], pt[:], AF.Square,
                                     accum_out=ssq[:])
                rt = work.tile([128, 1], F32, tag="rt")
                nc.scalar.activation(rt[:], ssq[:], AF.Sqrt, bias=eps_s[:],
                                     scale=1.0 / (KV * WSCALE * WSCALE))
                sg = work.tile([128, 1], F32, tag="sg", name=f"sg{g}", bufs=4)
                nc.vector.reciprocal(sg[:], rt[:])
                s_t[g] = sg

            def stats(g):
                stats_mm(g, 0, 8)
                stats_fin(g)

            def cast8(g):
                # fp16 -> fp8 stats copy for slab g from its x16 quarter.
                # Emission position matters: DVE is strict FIFO, so this
                # must be emitted only where its x16 piece is already due.
                q = g // 2
                t0 = (g % 2) * SLAB
                nc.vector.tensor_copy(x8_s[:, g, :, :],
                                      x16_s[q][:, :, t0:t0 + SLAB])

            def vraw(q):
                pv = ps_v.tile([128, 512], F32, tag="pv", name=f"pv{q}")
                for ck in range(NCK):
                    nc.tensor.matmul(pv[:, 0:QGRP], wf_s[:, ck, :],
                                     x16_s[q][:, ck, :],
                                     start=(ck == 0), stop=(ck == NCK - 1))
                dst = vts[:, q * QGRP:(q + 1) * QGRP]
                if with_ba:
                    nc.scalar.activation(dst, pv[:, 0:QGRP], AF.Identity,
                                         bias=vb_s[:], scale=1.0)
                else:
                    nc.vector.tensor_copy(dst, pv[:, 0:QGRP])

            def step4(g):
                t0 = g * SLAB
                ysb = work.tile([128, OUT], F16, tag="ysb", bufs=6)
                for n in range(4):
                    py = ps_y.tile([128, 512], F32, tag="py",
                                   name=f"py{g}_{n}")
                    nc.tensor.matmul(py[:], vts[:, t0:t0 + SLAB],
                                     mt_s[:, n * 512:(n + 1) * 512],
                                     start=True, stop=True)
                    ysl = ysb[:, n * 512:(n + 1) * 512]
                    if n % 2 == 0:
                        nc.vector.tensor_scalar_mul(ysl, py[:], s_t[g][:])
                    else:
                        nc.scalar.activation(ysl, py[:], AF.Identity,
                                             bias=0.0, scale=s_t[g][:])
                    if with_bv:
                        nc.vector.tensor_add(
                            ysl, ysl, crb_s[:, n * 512:(n + 1) * 512])
                    if g >= nslab - 3 and n == 1:
                        # tail slabs: first half fired as soon as it is
                        # scaled; earlier slabs ride scalar so they do
                        # not queue behind the input FIFO on sync
                        nc.sync.dma_start(y_d[t0:t0 + SLAB, 0:1024],
                                          ysb[:, 0:1024])
                if g >= nslab - 3:
                    nc.sync.dma_start(y_d[t0:t0 + SLAB, 1024:2048],
                                      ysb[:, 1024:2048])
                else:
                    nc.scalar.dma_start(y_d[t0:t0 + SLAB, :], ysb[:])

            # ---------------- PE emission order ----------------
            # slabs 0/1 staggered by w8 piece so PE starts on the first
            # 0.75 MiB; stats 2/3 fill the window while wf/xq0 stream in.
            stats_mm(0, 0, 2)
            stats_mm(1, 0, 2)
            stats_mm(0, 2, 5)
            stats_mm(1, 2, 5)
            stats_mm(0, 5, 8)
            stats_mm(1, 5, 8)
            stats_fin(0)
            stats_fin(1)
            stats(2)
            stats(3)
            vraw(0)
            step4(0)
            step4(1)
            vraw(1)
            step4(2)
            step4(3)
            stats(4)
            stats(5)
            vraw(2)
            step4(4)
            step4(5)
            stats(6)
            stats(7)
            vraw(3)
            step4(6)
            step4(7)

    nc.compile()
    return nc


def _host_prep(inputs):
    h = np.asarray(inputs["hidden_states"], dtype=np.float32)
    b, s, hid = h.shape
    assert hid == HID
    x = np.ascontiguousarray(h.reshape(b * s, hid))
    ntok = b * s
    tok = ntok // N_CORES
    nq = tok // QGRP

    kv_a_w = np.asarray(inputs["kv_a_w"], np.float64)
    kv_a_b = np.asarray(inputs["kv_a_b"], np.float64)
    kv_norm_w = np.asarray(inputs["kv_norm_w"], np.float64)
    kv_b_w = np.asarray(inputs["kv_b_w"], np.float64)
    kv_b_b = np.asarray(inputs["kv_b_b"], np.float64)
    o_w = np.asarray(inputs["o_w"], np.float64)

    wv = kv_b_w[D:2 * D] * (1.0 + kv_norm_w)[None, :]          # (128, 512)
    wf = wv @ kv_a_w                                           # (128, 2048)
    M = o_w.reshape(HID, 16, D).sum(axis=1)                    # (2048, 128)

    w8 = np.ascontiguousarray(
        (kv_a_w.T * WSCALE).reshape(NCK, 128, KV).transpose(1, 0, 2)
    ).astype(np.float32).astype(ml_dtypes.float8_e4m3)
    wf_sw = np.ascontiguousarray(
        wf.T.reshape(NCK, 128, D).transpose(1, 0, 2)).astype(np.float16)
    mt = np.ascontiguousarray(M.T).astype(np.float16)

    with_ba = bool(np.any(kv_a_b != 0.0))
    with_bv = bool(np.any(kv_b_b[D:2 * D] != 0.0))

    in_maps = []
    for i in range(N_CORES):
        shard = x[i * tok:(i + 1) * tok]                       # (tok, 2048)
        xt = shard.T.reshape(NCK, 128, tok)                    # hid-major
        ht = NHOST8 * SLAB
        x8 = np.ascontiguousarray(
            xt[:, :, :ht].reshape(NCK, 128, NHOST8 // 2, 2, SLAB)
            .transpose(2, 1, 3, 0, 4)
        ).astype(ml_dtypes.float8_e4m3)
        x16 = np.ascontiguousarray(
            xt.reshape(NCK, 128, nq, QGRP).transpose(2, 1, 0, 3)
        ).astype(np.float16)
        m = {"x8": x8, "x16": x16, "w8": w8, "wf": wf_sw, "mt": mt}
        if with_ba:
            m["bar"] = (kv_a_b.reshape(1, KV) * WSCALE).astype(
                np.float32).astype(ml_dtypes.float8_e4m3)
            m["ones8"] = np.ones((1, 128), np.float32).astype(
                ml_dtypes.float8_e4m3)
            m["vb"] = np.ascontiguousarray(
                (wv @ kv_a_b).reshape(D, 1)).astype(np.float32)
        if with_bv:
            cr = (M @ kv_b_b[D:2 * D]).reshape(1, OUT)
            m["crb"] = np.ascontiguousarray(
                np.broadcast_to(cr, (128, OUT))).astype(np.float16)
        in_maps.append(m)

    def gather(results):
        y = np.concatenate([r["y"] for r in results], axis=0)
        return np.ascontiguousarray(y.reshape(b, s, HID).astype(np.float32))

    return in_maps, gather, with_ba, with_bv, tok


def _run(inputs, trace=False, **spmd_kwargs):
    in_maps, gather, with_ba, with_bv, tok = _host_prep(inputs)
    key = (tok, with_ba, with_bv)
    if key not in _NC_CACHE:
        _NC_CACHE[key] = _build_nc(tok, with_ba, with_bv)
    nc = _NC_CACHE[key]
    res = run_bass_kernel_spmd(nc, in_maps, core_ids=list(range(N_CORES)),
                               trace=trace, **spmd_kwargs)
    return gather(res.results), res


def kernel(**inputs) -> np.ndarray:
    y, _ = _run(inputs, trace=False)
    return y
